# revision 5
# baseline (speedup 1.0000x reference)
"""Trainium2 Bass kernel for nn_Decompose_13477607375164.

The reference computation collapses to a per-image-plane 5x5 convolution:
    out = clip( sum_{i,j} w'[i,j] * clip(x,0,1)[.., r+i-2, c+j-2] + c', 0, 1 )
with reflect padding, where w'[i,j] = (wS_k . wE_k)/25 for k = i*5+j and
c' = (sum_k (wS_k . bE_k + bS_k)) / 25.

Strategy (pure data parallel over the 12 image planes, 8 cores):
  - Host: compute the 25 scalar taps + constant (tiny), reflect-pad each
    plane, hand each core 3 padded half-planes of (516, 1028) fp32.
  - Device: for each 128-row tile, the vertical taps are a banded-matrix
    matmul on the TensorEngine (stationary banded lhsT), the horizontal
    taps are free-dim shifts of the moving operand; 5 shift-matmuls
    accumulate in PSUM.  Precision: main product in fp32r (~tf32) plus two
    bf16 correction products (w_r*x_lo and w_e*x_hi), giving fp32-class
    accuracy at 1 cycle/row instead of fp32's 4.
  - The constant c' rides in band row 0 against an all-ones partition.
"""

import numpy as np
import ml_dtypes

import concourse.bacc as bacc
import concourse.mybir as mybir
from concourse.tile import TileContext
from concourse.bass_utils import run_bass_kernel_spmd

BS, C, H, W = 4, 3, 1024, 1024
SIZE = 5
PAD = 2
NCORES = 8
NSEG = 3            # half-planes per core
SEG_OUT = 512       # output rows per segment
SEG_IN = SEG_OUT + 2 * PAD    # 516
INCOLS = W + 2 * PAD          # 1028
KDIM = 128
MG = 123            # output rows per full row-group (127 x-rows + 1 const row)
GROUP_M0 = (0, 123, 246, 369, 492)
NCHUNK = 512

F32 = mybir.dt.float32
F32R = mybir.dt.float32r
BF16 = mybir.dt.bfloat16

_prog_cache = {}

# Number of on-device repetitions of the whole computation (used only for
# differential HW-time measurement from test.py; grading uses 1 = no loop).
REPEAT = 1


def _tf32_round(a: np.ndarray) -> np.ndarray:
    """Round fp32 to 10 explicit mantissa bits (RNE). Values produced here are
    exactly representable in the hardware fp32r format."""
    u = a.astype(np.float32).view(np.uint32).astype(np.uint64)
    half = np.uint64(0x0FFF) + ((u >> np.uint64(13)) & np.uint64(1))
    u = ((u + half) & np.uint64(0xFFFFE000)).astype(np.uint32)
    return u.view(np.float32)


def _build_program(repeat=1):
    nc = bacc.Bacc(None, target_bir_lowering=False, debug=True)
    xseg = nc.dram_tensor("xseg", [NSEG, SEG_IN, INCOLS], F32, kind="ExternalInput")
    br = nc.dram_tensor("br", [KDIM, 5 * MG], F32R, kind="ExternalInput")
    blo = nc.dram_tensor("blo", [KDIM, 5 * MG], BF16, kind="ExternalInput")
    bwe = nc.dram_tensor("bwe", [KDIM, 5 * MG], BF16, kind="ExternalInput")
    y = nc.dram_tensor("y", [NSEG, SEG_OUT, W], F32, kind="ExternalOutput")

    from contextlib import ExitStack

    with TileContext(nc) as tc:
        with (
            tc.tile_pool(name="wconst", bufs=1) as cpool,
            tc.tile_pool(name="xp", bufs=3) as xpool,
            tc.tile_pool(name="op", bufs=4) as opool,
            tc.tile_pool(name="ps", bufs=4, space="PSUM") as pspool,
            ExitStack() as stack,
        ):
            brt = cpool.tile([KDIM, 5 * MG], F32R)
            blot = cpool.tile([KDIM, 5 * MG], BF16)
            bwet = cpool.tile([KDIM, 5 * MG], BF16)
            nc.sync.dma_start(out=brt[:, :], in_=br[:, :])
            nc.sync.dma_start(out=blot[:, :], in_=blo[:, :])
            nc.sync.dma_start(out=bwet[:, :], in_=bwe[:, :])

            if repeat > 1:
                stack.enter_context(
                    tc.For_i(
                        0, repeat, 1,
                        hint_engines=(
                            mybir.EngineType.PE,
                            mybir.EngineType.DVE,
                            mybir.EngineType.Activation,
                            mybir.EngineType.SP,
                        ),
                    )
                )

            for s in range(NSEG):
                for m0 in GROUP_M0:
                    nrows = min(KDIM - 1, SEG_IN - m0)   # 127 or 24
                    kdim = nrows + 1
                    mg = min(MG, SEG_OUT - m0)           # 123 or 20

                    xraw = xpool.tile([KDIM, INCOLS], F32, tag="xraw")
                    nc.vector.memset(xraw[0:1, :], 1.0)
                    nc.sync.dma_start(
                        out=xraw[1:1 + nrows, :], in_=xseg[s, m0:m0 + nrows, :]
                    )
                    t32 = xpool.tile([KDIM, INCOLS], F32, tag="t32")
                    xhi = xpool.tile([KDIM, INCOLS], F32R, tag="xhi")
                    xhib = xpool.tile([KDIM, INCOLS], BF16, tag="xhib")
                    xlob = xpool.tile([KDIM, INCOLS], BF16, tag="xlob")

                    # t32 = clip(x, 0, 1); xhi = fp32r(t32); xlo = t32 - xhi
                    nc.vector.tensor_scalar(
                        t32[:, :], xraw[:, :], 0.0, 1.0,
                        mybir.AluOpType.max, mybir.AluOpType.min,
                    )
                    nc.scalar.copy(xhi[:, :], t32[:, :])
                    nc.scalar.copy(xhib[:, :], xhi[:, :])
                    nc.vector.tensor_tensor(
                        xlob[:, :], t32[:, :], xhi[:, :].bitcast(F32),
                        mybir.AluOpType.subtract,
                    )

                    for n0 in (0, NCHUNK):
                        ps = pspool.tile([KDIM, NCHUNK], F32, tag="ps")
                        # bf16 corrections first (they can carry the sync
                        # waits; the fp32r matmuls then need none).
                        for j in range(SIZE):
                            nc.tensor.matmul(
                                ps[0:mg, :],
                                blot[0:kdim, j * MG:j * MG + mg],
                                xlob[0:kdim, n0 + j:n0 + j + NCHUNK],
                                start=(j == 0), stop=False,
                            )
                        for j in range(SIZE):
                            nc.tensor.matmul(
                                ps[0:mg, :],
                                bwet[0:kdim, j * MG:j * MG + mg],
                                xhib[0:kdim, n0 + j:n0 + j + NCHUNK],
                                start=False, stop=False,
                            )
                        for j in range(SIZE):
                            nc.tensor.matmul(
                                ps[0:mg, :],
                                brt[0:kdim, j * MG:j * MG + mg],
                                xhi[0:kdim, n0 + j:n0 + j + NCHUNK],
                                start=False, stop=(j == SIZE - 1),
                            )
                        ot = opool.tile([KDIM, NCHUNK], F32, tag="ot")
                        nc.vector.tensor_scalar(
                            ot[0:mg, :], ps[0:mg, :], 0.0, 1.0,
                            mybir.AluOpType.max, mybir.AluOpType.min,
                        )
                        nc.sync.dma_start(
                            out=y[s, m0:m0 + mg, n0:n0 + NCHUNK], in_=ot[0:mg, :]
                        )
    nc.compile()
    return nc


def _build_weights(wE, bE, wS, bS):
    a = np.einsum("kd,kd->k", wS.astype(np.float64), wE.astype(np.float64))
    cvec = np.einsum("kd,kd->k", wS.astype(np.float64), bE.astype(np.float64)) \
        + bS.astype(np.float64)
    # match the reference's fp32 arithmetic for the tap values
    a32 = np.einsum("kd,kd->k", wS, wE).astype(np.float32)
    c32 = (np.einsum("kd,kd->k", wS, bE).astype(np.float32)
           + bS.astype(np.float32)).astype(np.float32)
    del a, cvec
    wp = (a32 / np.float32(SIZE * SIZE)).astype(np.float32).reshape(SIZE, SIZE)
    cprime = np.float32(c32.sum(dtype=np.float32) / np.float32(SIZE * SIZE))

    w_r = _tf32_round(wp)
    w_e = (wp - w_r).astype(np.float32)

    br = np.zeros((KDIM, SIZE, MG), np.float32)
    blo = np.zeros((KDIM, SIZE, MG), np.float32)
    bwe = np.zeros((KDIM, SIZE, MG), np.float32)
    for i in range(SIZE):
        # band: out row m uses x row m+i, stored at partition 1+m+i
        for j in range(SIZE):
            kk = np.arange(MG) + 1 + i
            br[kk, j, np.arange(MG)] = w_r[i, j]
            blo[kk, j, np.arange(MG)] = w_r[i, j]
            bwe[kk, j, np.arange(MG)] = w_e[i, j]
    cr = _tf32_round(np.array([cprime], np.float32))[0]
    br[0, 0, :] = cr
    bwe[0, 0, :] = np.float32(cprime - cr)

    return (
        br.reshape(KDIM, SIZE * MG),
        blo.reshape(KDIM, SIZE * MG).astype(ml_dtypes.bfloat16),
        bwe.reshape(KDIM, SIZE * MG).astype(ml_dtypes.bfloat16),
    )


def kernel(x, wE, bE, wS, bS, _trace=False):
    x = np.asarray(x, dtype=np.float32)
    planes = x.reshape(BS * C, H, W)
    xp = np.pad(planes, ((0, 0), (PAD, PAD), (PAD, PAD)), mode="reflect")

    br, blo, bwe = _build_weights(
        np.asarray(wE, np.float32), np.asarray(bE, np.float32),
        np.asarray(wS, np.float32), np.asarray(bS, np.float32),
    )

    in_maps = []
    for core in range(NCORES):
        segs = np.empty((NSEG, SEG_IN, INCOLS), np.float32)
        for k in range(NSEG):
            h = core * NSEG + k          # half-plane index 0..23
            p, half = divmod(h, 2)
            segs[k] = xp[p, half * SEG_OUT: half * SEG_OUT + SEG_IN, :]
        in_maps.append({"xseg": segs, "br": br, "blo": blo, "bwe": bwe})

    key = ("prog", REPEAT)
    if key not in _prog_cache:
        _prog_cache[key] = _build_program(REPEAT)
    nc = _prog_cache[key]

    res = run_bass_kernel_spmd(
        nc, in_maps, core_ids=list(range(NCORES)), trace=bool(_trace)
    )

    out = np.empty((BS * C, H, W), np.float32)
    for core in range(NCORES):
        yc = res.results[core]["y"]
        for k in range(NSEG):
            h = core * NSEG + k
            p, half = divmod(h, 2)
            out[p, half * SEG_OUT:(half + 1) * SEG_OUT, :] = yc[k]
    out = out.reshape(BS, C, H, W)

    if _trace:
        return out, res
    return out
